# revision 24
# baseline (speedup 1.0000x reference)
"""TTT linear layer (B=2, S=256, H=768) on 8 TRN2 NeuronCores.

Strategy:
- Delta-rule reformulation: the sequential fast-weight recurrence is a unit
  lower-triangular solve (I + N) D = RHS per batch, with
  N = lr * strict_tril(X X^T + 1), RHS = X W0^T + b0 - X_next, pred = D + X_next.
  Solved in 2 chunks of 128 via explicit inverses from the product formula
  inv(I+U) = (I+U^32)...(I+U^2)(I-U)  (exact enough for nilpotent U, validated).
- Sharding: core c = (batch b=c//4, FFN quarter q=c%4). Each core solves its
  batch's recurrence (replicated x4), computes gate/up/down for its 768 FFN
  dims over the batch's 256 tokens, adds (pred + down_b)/4, then one
  AllToAll over groups [[0..3],[4..7]] routes 64-token shards; each core sums
  its 4 shards, applies LayerNorm, and writes its 64-token output slice.
"""

import ml_dtypes
import numpy as np

import concourse.bass as bass
import concourse.mybir as mybir
import concourse.tile as tile
from concourse import bass_utils

# ----------------------------------------------------------------------------
# Workaround: walrus in this container accepts only ONE sync-wait per
# instruction. Tile emits multi-wait instructions routinely. Post-pass: hoist
# all but the last wait of each instruction onto nofuse nops inserted just
# before it on the same engine queue (queue ordering preserves semantics).
def _split_multi_waits(nc):
    idx = 0
    for f in nc.m.functions:
        for bb in f.blocks:
            il = bb.instructions
            i = 0
            while i < len(il):
                ins = il[i]
                si = ins.sync_info
                if si is not None and len(si.on_wait) > 1:
                    waits = list(si.on_wait)
                    for w in waits[:-1]:
                        nop = mybir.InstNoOp(
                            name=f"waitsplit-{idx}",
                            engine=ins.engine,
                            bass_nofuse=True,
                            sync_info=mybir.SyncInfo(on_wait=[w], on_update=[]),
                        )
                        idx += 1
                        il.insert(i, nop)
                        i += 1
                    ins.sync_info = mybir.SyncInfo(
                        on_wait=[waits[-1]], on_update=list(si.on_update)
                    )
                i += 1
    return idx


_SLIMMED = False


def _slim_tile_exit():
    """Drop the final all-engine barrier of the Tile exit sequence (the drain
    waits + first barrier already prove all work complete; gpsimd's sem clears
    are then the last ops before program end)."""
    global _SLIMMED
    if _SLIMMED:
        return
    _SLIMMED = True

    def _drain_and_barrier(self, tick_clock, wait_clock):
        nc = self.nc
        drain_inst = nc.sync.drain()
        wait_clock.add_sem_waits(
            drain_inst.ins, tile.ScopedClock({None: tick_clock.global_clock})
        )
        nc.all_engine_barrier()
        popped = nc._tile_sem_poison_stack.pop()
        assert popped is self._sem_poison
        nc.clear_and_free_semaphores(list(self.sems.allocated().values()))

    tile.TileContext._drain_and_barrier = _drain_and_barrier


# ----------------------------------------------------------------------------
B, S, H = 2, 256, 768
F = 4 * H            # 3072
FQ = F // 4          # 768 FFN dims per core
KH = H // 128        # 6 H-chunks
NF = FQ // 128       # 6 FFN tiles per core
LR = 0.01
EPS = 1e-5
NSTEP = 5            # inverse product-formula steps (validated: rel err 4e-6)

F32 = mybir.dt.float32
F32R = mybir.dt.float32r
BF16 = mybir.dt.bfloat16
AF = mybir.ActivationFunctionType


def _f(ap):
    """View a float32r AP as plain fp32 for DVE/ACT consumers (same bits)."""
    return ap.bitcast(F32)

TRACE = False
TRACE_KW = {}


def build_bass():
    _slim_tile_exit()
    nc = bass.Bass()
    dt = F32
    xt = nc.dram_tensor("xt", [128, KH, S], F32R, kind="ExternalInput")        # x[b].T
    xtok = nc.dram_tensor("xtok", [S, H], dt, kind="ExternalInput")    # x[b]
    w0t = nc.dram_tensor("w0t", [128, KH, H], F32R, kind="ExternalInput")      # W_init.T
    gwt = nc.dram_tensor("gwt", [128, KH, FQ], BF16, kind="ExternalInput")     # gate_w[fsl].T
    uwt = nc.dram_tensor("uwt", [128, KH, FQ], BF16, kind="ExternalInput")     # up_w[fsl].T
    dwt = nc.dram_tensor("dwt", [128, NF, H], BF16, kind="ExternalInput")     # down_w[:,fsl].T
    gb = nc.dram_tensor("gb", [128, NF], dt, kind="ExternalInput")
    ub = nc.dram_tensor("ub", [128, NF], dt, kind="ExternalInput")
    b0 = nc.dram_tensor("b0", [H], dt, kind="ExternalInput")
    db = nc.dram_tensor("db", [H], dt, kind="ExternalInput")
    nw = nc.dram_tensor("nw", [H], dt, kind="ExternalInput")
    nbt = nc.dram_tensor("nbt", [H], dt, kind="ExternalInput")
    lowm = nc.dram_tensor("lowm", [128, 128], dt, kind="ExternalInput")
    upm = nc.dram_tensor("upm", [128, 128], dt, kind="ExternalInput")
    eye = nc.dram_tensor("eye", [128, 128], dt, kind="ExternalInput")
    z0 = nc.dram_tensor("z0", [128, 1], dt, kind="ExternalInput")
    z1 = nc.dram_tensor("z1", [128, 1], dt, kind="ExternalInput")
    z0q = nc.dram_tensor("z0q", [128, 1], dt, kind="ExternalInput")
    z1q = nc.dram_tensor("z1q", [128, 1], dt, kind="ExternalInput")
    out = nc.dram_tensor("out", [64, H], dt, kind="ExternalOutput")

    with tile.TileContext(nc) as tc:
        with tc.tile_pool(name="singles", bufs=1) as singles, \
             tc.tile_pool(name="work", bufs=2) as work, \
             tc.tile_pool(name="dram", bufs=1, space="DRAM") as dram:

            # --------------------------------------------------------------
            # Loads: ordered by when compute needs them (critical first).
            xt_sb = singles.tile([128, KH, S], F32R)
            nc.sync.dma_start(xt_sb[:], xt[:, :, :])
            lowm_sb = singles.tile([128, 128], F32)
            nc.sync.dma_start(lowm_sb[:], lowm[:, :])
            upm_sb = singles.tile([128, 128], F32)
            nc.sync.dma_start(upm_sb[:], upm[:, :])
            eye_sb = singles.tile([128, 128], F32)
            nc.sync.dma_start(eye_sb[:], eye[:, :])
            w0t_sb = singles.tile([128, KH, H], F32R)
            nc.sync.dma_start(w0t_sb[:], w0t[:, :, :])
            xn_sb = singles.tile([128, 2, H], F32)
            nc.vector.memset(xn_sb[:, 1, :], 0.0)
            nc.sync.dma_start(xn_sb[:, 0, :], xtok[1:129, :])
            nc.sync.dma_start(xn_sb[:127, 1, :], xtok[129:256, :])
            b0_bc = singles.tile([128, H], F32)
            nc.sync.dma_start(b0_bc[:], b0[:].partition_broadcast(128))
            db_bc = singles.tile([128, H], F32)
            nc.sync.dma_start(db_bc[:], db[:].partition_broadcast(128))
            z0_sb = singles.tile([128, 1], F32)
            nc.sync.dma_start(z0_sb[:], z0[:, :])
            z1_sb = singles.tile([128, 1], F32)
            nc.sync.dma_start(z1_sb[:], z1[:, :])
            z0q_sb = singles.tile([128, 1], F32)
            nc.sync.dma_start(z0q_sb[:], z0q[:, :])
            z1q_sb = singles.tile([128, 1], F32)
            nc.sync.dma_start(z1q_sb[:], z1q[:, :])
            gb_sb = singles.tile([128, NF], F32)
            nc.sync.dma_start(gb_sb[:], gb[:, :])
            ub_sb = singles.tile([128, NF], F32)
            nc.sync.dma_start(ub_sb[:], ub[:, :])
            nw_bc = singles.tile([64, H], F32)
            nc.sync.dma_start(nw_bc[:], nw[:].partition_broadcast(64))
            nb_bc = singles.tile([64, H], F32)
            nc.sync.dma_start(nb_bc[:], nbt[:].partition_broadcast(64))
            gwt_sb = singles.tile([128, KH, FQ], BF16)
            nc.sync.dma_start(gwt_sb[:], gwt[:, :, :])
            uwt_sb = singles.tile([128, KH, FQ], BF16)
            nc.sync.dma_start(uwt_sb[:], uwt[:, :, :])
            dwt_sb = singles.tile([128, NF, H], BF16)
            nc.sync.dma_start(dwt_sb[:], dwt[:, :, :])

            # warm up the collectives engine early with a tiny RS so the real
            # one at the end does not pay ncfw first-call startup
            warm_in = dram.tile([8, 16], F32)
            warm_out = dram.tile([1, 16], F32)
            warm_sb = work.tile([8, 16], F32, tag="warm", bufs=1)
            nc.vector.memset(warm_sb[:], 0.0)
            nc.sync.dma_start(warm_in[:, :], warm_sb[:])
            nc.gpsimd.collective_compute(
                "ReduceScatter",
                mybir.AluOpType.add,
                replica_groups=[[0, 1, 2, 3, 4, 5, 6, 7]],
                ins=[warm_in[:, :]],
                outs=[warm_out[:, :]],
            )

            lr_bias = singles.tile([128, 1], F32)
            nc.vector.memset(lr_bias[:], LR)
            lrG_A = singles.tile([128, 256], F32R)
            lrG_B = singles.tile([128, 128], F32R)
            Qlo = singles.tile([128, 2, 128], F32R)
            Qup = singles.tile([128, 2, 128], F32R)
            Tt = singles.tile([128, 2, 128], F32R)
            R = singles.tile([128, 2, H], F32R)
            d_tok = singles.tile([128, 2, H], F32R)
            predT = singles.tile([128, KH, S], BF16)
            pred_q = singles.tile([128, 2, H], F32)
            pred_q0 = singles.tile([128, 2, H], F32)
            pred_q1 = singles.tile([128, 2, H], F32)

            with tc.tile_pool(name="psum_p1", bufs=3, space="PSUM") as psum_p1:
                # ----------------------------------------------------------
                # Phase 1a: Gram blocks -> lr*(G+1)
                ps_ga = psum_p1.tile([128, 256], F32, tag="p1")
                for k in range(KH):
                    nc.tensor.matmul(ps_ga[:], (xt_sb[:, k, 0:128]),
                                     (xt_sb[:, k, :]),
                                     start=(k == 0), stop=(k == KH - 1))
                nc.scalar.activation(lrG_A[:], ps_ga[:], AF.Identity,
                                     bias=lr_bias[:], scale=LR)
                ps_gb = psum_p1.tile([128, 128], F32, tag="p1")
                for k in range(KH):
                    nc.tensor.matmul(ps_gb[:], xt_sb[:, k, 128:256],
                                     xt_sb[:, k, 128:256],
                                     start=(k == 0), stop=(k == KH - 1))
                nc.scalar.activation(lrG_B[:], ps_gb[:], AF.Identity,
                                     bias=lr_bias[:], scale=LR)

                nc.vector.tensor_mul(Qlo[:, 0, :], _f(lrG_A[:, 0:128]), lowm_sb[:])
                nc.vector.tensor_mul(Qup[:, 0, :], _f(lrG_A[:, 0:128]), upm_sb[:])
                nc.vector.tensor_mul(Qlo[:, 1, :], _f(lrG_B[:]), lowm_sb[:])
                nc.vector.tensor_mul(Qup[:, 1, :], _f(lrG_B[:]), upm_sb[:])
                nc.vector.tensor_sub(Tt[:, 0, :], eye_sb[:], _f(Qup[:, 0, :]))
                nc.vector.tensor_sub(Tt[:, 1, :], eye_sb[:], _f(Qup[:, 1, :]))

                # Phase 1b: Tt_c = inv(I + U_c), product formula, both chunks
                for s in range(1, NSTEP + 1):
                    ps_l = psum_p1.tile([128, 2, 128], F32, tag="p1")
                    ps_u = psum_p1.tile([128, 2, 128], F32, tag="p1")
                    for c in range(2):
                        nc.tensor.matmul(ps_l[:, c, :], _f(Qup[:, c, :]),
                                         _f(Qlo[:, c, :]), start=True, stop=True)
                        if s < NSTEP:
                            nc.tensor.matmul(ps_u[:, c, :], _f(Qlo[:, c, :]),
                                             _f(Qup[:, c, :]), start=True, stop=True)
                    nc.vector.tensor_copy(Qlo[:], ps_l[:])
                    if s < NSTEP:
                        nc.vector.tensor_copy(Qup[:], ps_u[:])
                    ps_t = psum_p1.tile([128, 2, 128], F32, tag="p1")
                    for c in range(2):
                        nc.tensor.matmul(ps_t[:, c, :], _f(Qlo[:, c, :]),
                                         _f(Tt[:, c, :]), start=True, stop=True)
                    nc.vector.tensor_add(Tt[:], _f(Tt[:]), ps_t[:])

                # ----------------------------------------------------------
                # Phase 1c: P0 (token-major), R = P0 + b0 - xnext
                for c in range(2):
                    ps_p0 = psum_p1.tile([128, H], F32, tag="p1")
                    for k in range(KH):
                        nc.tensor.matmul(ps_p0[:, 0:512],
                                         (xt_sb[:, k, bass.ts(c, 128)]),
                                         (w0t_sb[:, k, 0:512]),
                                         start=(k == 0), stop=(k == KH - 1))
                        nc.tensor.matmul(ps_p0[:, 512:768],
                                         (xt_sb[:, k, bass.ts(c, 128)]),
                                         (w0t_sb[:, k, 512:768]),
                                         start=(k == 0), stop=(k == KH - 1))
                    nc.vector.tensor_sub(R[:, c, :], ps_p0[:], xn_sb[:, c, :])
                    nc.vector.tensor_add(R[:, c, :], R[:, c, :], b0_bc[:])

                # Phase 1d: solve
                ps_d1 = psum_p1.tile([128, H], F32, tag="p1")
                nc.tensor.matmul(ps_d1[:, 0:512], (Tt[:, 0, :]),
                                 (R[:, 0, 0:512]), start=True, stop=True)
                nc.tensor.matmul(ps_d1[:, 512:768], (Tt[:, 0, :]),
                                 (R[:, 0, 512:768]), start=True, stop=True)
                nc.vector.tensor_copy(d_tok[:, 0, :], ps_d1[:])
                ps_v = psum_p1.tile([128, H], F32, tag="p1")
                nc.tensor.matmul(ps_v[:, 0:512], (lrG_A[:, 128:256]),
                                 (d_tok[:, 0, 0:512]), start=True, stop=True)
                nc.tensor.matmul(ps_v[:, 512:768], (lrG_A[:, 128:256]),
                                 (d_tok[:, 0, 512:768]), start=True, stop=True)
                nc.vector.tensor_sub(R[:, 1, :], _f(R[:, 1, :]), ps_v[:])
                ps_d2 = psum_p1.tile([128, H], F32, tag="p1")
                nc.tensor.matmul(ps_d2[:, 0:512], (Tt[:, 1, :]),
                                 (R[:, 1, 0:512]), start=True, stop=True)
                nc.tensor.matmul(ps_d2[:, 512:768], (Tt[:, 1, :]),
                                 (R[:, 1, 512:768]), start=True, stop=True)
                nc.vector.tensor_copy(d_tok[:, 1, :], ps_d2[:])

                # pred_q{0,1} = (pred + db)*0.25*z{0,1} (token-major)
                for c in range(2):
                    nc.vector.tensor_add(pred_q[:, c, :], _f(d_tok[:, c, :]),
                                         xn_sb[:, c, :])
                    nc.vector.tensor_add(pred_q[:, c, :], pred_q[:, c, :], db_bc[:])
                    nc.scalar.activation(pred_q0[:, c, :], pred_q[:, c, :],
                                         AF.Identity, scale=z0q_sb[:])
                    nc.scalar.activation(pred_q1[:, c, :], pred_q[:, c, :],
                                         AF.Identity, scale=z1q_sb[:])

                # feat-major predT = (R^T Tt) + shifted x^T
                for c in range(2):
                    for k in range(KH):
                        ps_dt = psum_p1.tile([128, 128], F32, tag="p1")
                        nc.tensor.matmul(ps_dt[:], R[:, c, bass.ts(k, 128)],
                                         Tt[:, c, :], start=True, stop=True)
                        if c == 0:
                            nc.vector.tensor_add(predT[:, k, 0:128], ps_dt[:],
                                                 _f(xt_sb[:, k, 1:129]))
                        else:
                            nc.vector.tensor_add(predT[:, k, 128:255],
                                                 ps_dt[:, 0:127],
                                                 _f(xt_sb[:, k, 129:256]))
                            nc.vector.tensor_copy(predT[:, k, 255:256],
                                                  ps_dt[:, 127:128])

            # --------------------------------------------------------------
            # Phase 2: MLP (feat-major gate/up, token-major down partials)
            with tc.tile_pool(name="psum_dn", bufs=1, space="PSUM") as psum_dn, \
                 tc.tile_pool(name="psum_mlp", bufs=4, space="PSUM") as psum_mlp:
                ps_dn0 = psum_dn.tile([128, H], F32, tag="dn0")
                ps_dn1 = psum_dn.tile([128, H], F32, tag="dn1")
                ps_dns = (ps_dn0, ps_dn1)
                for j in range(NF):
                    ps_g = psum_mlp.tile([128, S], F32, tag="mlp")
                    ps_u = psum_mlp.tile([128, S], F32, tag="mlp")
                    for k in range(KH):
                        nc.tensor.matmul(ps_g[:], (gwt_sb[:, k, bass.ts(j, 128)]),
                                         (predT[:, k, :]),
                                         start=(k == 0), stop=(k == KH - 1))
                    for k in range(KH):
                        nc.tensor.matmul(ps_u[:], (uwt_sb[:, k, bass.ts(j, 128)]),
                                         (predT[:, k, :]),
                                         start=(k == 0), stop=(k == KH - 1))
                    sig = work.tile([128, S], F32, tag="sig")
                    nc.scalar.activation(sig[:], ps_g[:], AF.Sigmoid,
                                         bias=gb_sb[:, j : j + 1], scale=1.0)
                    gtd = work.tile([128, S], BF16, tag="gtd")
                    nc.vector.tensor_scalar(gtd[:], ps_u[:], ub_sb[:, j : j + 1],
                                            None, mybir.AluOpType.add)
                    nc.vector.tensor_mul(gtd[:], gtd[:], sig[:])
                    for m in range(2):
                        nc.tensor.matmul(ps_dns[m][:, 0:512],
                                         (gtd[:, bass.ts(m, 128)]),
                                         (dwt_sb[:, j, 0:512]),
                                         start=(j == 0), stop=(j == NF - 1))
                        nc.tensor.matmul(ps_dns[m][:, 512:768],
                                         (gtd[:, bass.ts(m, 128)]),
                                         (dwt_sb[:, j, 512:768]),
                                         start=(j == 0), stop=(j == NF - 1))

                # partial + (pred + db)/4  -> DRAM for the exchange.
                # 8-rank ReduceScatter over the full flattened token space
                # [B*S, H]: core c's tokens are exactly shard c (64c..64c+64).
                # Each core contributes its batch's partials in that batch's
                # row-half and zeros in the other; which half is selected by
                # the 0/1 input scalars z0/z1 (program stays SPMD-uniform).
                HP = 1024  # pad H so the 2S x HP bf16 buffer exceeds the
                # ~1MB algorithm crossover (mesh below is much slower here)
                rs_in = dram.tile([2 * S, HP], BF16)
                rs_out = dram.tile([64, HP], BF16)
                dp0 = work.tile([128, 2, HP], BF16, tag="dp0", bufs=1)
                dp1 = work.tile([128, 2, HP], BF16, tag="dp1", bufs=1)
                nc.vector.memset(dp0[:, :, H:], 0.0)
                nc.vector.memset(dp1[:, :, H:], 0.0)
                for m in range(2):
                    nc.scalar.activation(dp0[:, m, :H], ps_dns[m][:], AF.Identity,
                                         scale=z0_sb[:])
                    nc.vector.tensor_add(dp0[:, m, :H], dp0[:, m, :H],
                                         pred_q0[:, m, :])
                    nc.scalar.activation(dp1[:, m, :H], ps_dns[m][:], AF.Identity,
                                         scale=z1_sb[:])
                    nc.vector.tensor_add(dp1[:, m, :H], dp1[:, m, :H],
                                         pred_q1[:, m, :])
                    nc.sync.dma_start(rs_in[bass.ts(m, 128), :], dp0[:, m, :])
                    nc.sync.dma_start(rs_in[bass.ds(S + m * 128, 128), :],
                                      dp1[:, m, :])

            nc.gpsimd.collective_compute(
                "ReduceScatter",
                mybir.AluOpType.add,
                replica_groups=[[0, 1, 2, 3, 4, 5, 6, 7]],
                ins=[rs_in[:, :]],
                outs=[rs_out[:, :]],
            )

            # --------------------------------------------------------------
            # Phase 3: LayerNorm on the summed 64-token shard
            acc_h = work.tile([64, HP], BF16, tag="acc_h", bufs=1)
            nc.sync.dma_start(acc_h[:], rs_out[:, :])
            acc = work.tile([64, H], F32, tag="acc", bufs=1)
            nc.vector.tensor_copy(acc[:], acc_h[:, :H])

            nstats = nc.vector.BN_STATS_DIM
            stats = work.tile([64, 3, nstats], F32, tag="stats", bufs=1)
            accr = acc[:].rearrange("t (n d) -> t n d", d=256)
            for g in range(3):
                nc.vector.bn_stats(out=stats[:, g, :], in_=accr[:, g, :])
            mv = work.tile([64, nc.vector.BN_AGGR_DIM], F32, tag="mv", bufs=1)
            nc.vector.bn_aggr(out=mv[:], in_=stats[:])
            eps_t = work.tile([64, 1], F32, tag="eps", bufs=1)
            nc.vector.memset(eps_t[:], EPS)
            rstd = work.tile([64, 1], F32, tag="rstd", bufs=1)
            nc.scalar.activation(out=rstd[:], in_=mv[:, 1:2], func=AF.Sqrt,
                                 bias=eps_t[:], scale=1.0)
            nc.vector.reciprocal(out=rstd[:], in_=rstd[:])
            nc.vector.tensor_scalar(acc[:], acc[:], mv[:, 0:1], rstd[:],
                                    mybir.AluOpType.subtract,
                                    mybir.AluOpType.mult)
            nc.vector.tensor_mul(acc[:], acc[:], nw_bc[:])
            nc.vector.tensor_add(acc[:], acc[:], nb_bc[:])
            nc.sync.dma_start(out[:, :], acc[:])
    _split_multi_waits(nc)
    return nc


# ----------------------------------------------------------------------------
_NC_CACHE = None


def _get_nc():
    global _NC_CACHE
    if _NC_CACHE is None:
        _NC_CACHE = build_bass()
    return _NC_CACHE


def kernel(x, W_init, b_init, gate_w, gate_b, up_w, up_b, down_w, down_b,
           norm_w, norm_b):
    x = np.asarray(x, np.float32)
    W_init = np.asarray(W_init, np.float32)
    b_init = np.asarray(b_init, np.float32)
    gate_w = np.asarray(gate_w, np.float32)
    gate_b = np.asarray(gate_b, np.float32)
    up_w = np.asarray(up_w, np.float32)
    up_b = np.asarray(up_b, np.float32)
    down_w = np.asarray(down_w, np.float32)
    down_b = np.asarray(down_b, np.float32)
    norm_w = np.asarray(norm_w, np.float32)
    norm_b = np.asarray(norm_b, np.float32)

    nc = _get_nc()

    def shuf(a):
        # [O*128, N] -> [128, O, N] with partition p holding rows {o*128+p}
        o = a.shape[0] // 128
        return np.ascontiguousarray(a.reshape(o, 128, -1).transpose(1, 0, 2))

    lowmask = np.tril(np.ones((128, 128), np.float32), -1)
    upmask = np.triu(np.ones((128, 128), np.float32), 1)
    eye = np.eye(128, dtype=np.float32)
    w0t = shuf(W_init.T)

    in_maps = []
    for c in range(8):
        b, q = c // 4, c % 4
        fsl = slice(q * FQ, (q + 1) * FQ)
        in_maps.append({
            "xt": shuf(x[b].T),
            "xtok": np.ascontiguousarray(x[b]),
            "w0t": w0t,
            "gwt": shuf(gate_w[fsl].T.astype(ml_dtypes.bfloat16)),
            "uwt": shuf(up_w[fsl].T.astype(ml_dtypes.bfloat16)),
            "dwt": shuf(down_w[:, fsl].T.astype(ml_dtypes.bfloat16)),
            "gb": np.ascontiguousarray(gate_b[fsl].reshape(NF, 128).T),
            "ub": np.ascontiguousarray(up_b[fsl].reshape(NF, 128).T),
            "b0": b_init,
            "db": down_b,
            "nw": norm_w,
            "nbt": norm_b,
            "z0": np.full((128, 1), 1.0 if b == 0 else 0.0, np.float32),
            "z1": np.full((128, 1), 1.0 if b == 1 else 0.0, np.float32),
            "z0q": np.full((128, 1), 0.25 if b == 0 else 0.0, np.float32),
            "z1q": np.full((128, 1), 0.25 if b == 1 else 0.0, np.float32),
            "lowm": lowmask,
            "upm": upmask,
            "eye": eye,
        })

    res = bass_utils.run_bass_kernel_spmd(
        nc, in_maps, core_ids=list(range(8)), trace=TRACE, **TRACE_KW
    )
    if TRACE:
        kernel.last_exec_time_ns = res.exec_time_ns

    out = np.empty((B, S, H), np.float32)
    for c in range(8):
        b, q = c // 4, c % 4
        out[b, q * 64 : (q + 1) * 64, :] = res.results[c]["out"]
    return out


kernel.last_exec_time_ns = None


# revision 26
# speedup vs baseline: 1.1551x; 1.1551x over previous
"""TTT linear layer (B=2, S=256, H=768) on 8 TRN2 NeuronCores.

Strategy:
- Delta-rule reformulation: the sequential fast-weight recurrence is a unit
  lower-triangular solve (I + N) D = RHS per batch, with
  N = lr * strict_tril(X X^T + 1), RHS = X W0^T + b0 - X_next, pred = D + X_next.
  Solved in 2 chunks of 128 via explicit inverses from the product formula
  inv(I+U) = (I+U^32)...(I+U^2)(I-U)  (exact enough for nilpotent U, validated).
- Sharding: core c = (batch b=c//4, FFN quarter q=c%4). Each core solves its
  batch's recurrence (replicated x4), computes gate/up/down for its 768 FFN
  dims over the batch's 256 tokens, adds (pred + down_b)/4, then one
  AllToAll over groups [[0..3],[4..7]] routes 64-token shards; each core sums
  its 4 shards, applies LayerNorm, and writes its 64-token output slice.
"""

import ml_dtypes
import numpy as np

import concourse.bass as bass
import concourse.mybir as mybir
import concourse.tile as tile
from concourse import bass_utils

# ----------------------------------------------------------------------------
# Workaround: walrus in this container accepts only ONE sync-wait per
# instruction. Tile emits multi-wait instructions routinely. Post-pass: hoist
# all but the last wait of each instruction onto nofuse nops inserted just
# before it on the same engine queue (queue ordering preserves semantics).
def _split_multi_waits(nc):
    idx = 0
    for f in nc.m.functions:
        for bb in f.blocks:
            il = bb.instructions
            i = 0
            while i < len(il):
                ins = il[i]
                si = ins.sync_info
                if si is not None and len(si.on_wait) > 1:
                    waits = list(si.on_wait)
                    for w in waits[:-1]:
                        nop = mybir.InstNoOp(
                            name=f"waitsplit-{idx}",
                            engine=ins.engine,
                            bass_nofuse=True,
                            sync_info=mybir.SyncInfo(on_wait=[w], on_update=[]),
                        )
                        idx += 1
                        il.insert(i, nop)
                        i += 1
                    ins.sync_info = mybir.SyncInfo(
                        on_wait=[waits[-1]], on_update=list(si.on_update)
                    )
                i += 1
    return idx


_SLIMMED = False


def _slim_tile_exit():
    """Drop the final all-engine barrier of the Tile exit sequence (the drain
    waits + first barrier already prove all work complete; gpsimd's sem clears
    are then the last ops before program end)."""
    global _SLIMMED
    if _SLIMMED:
        return
    _SLIMMED = True

    def _drain_and_barrier(self, tick_clock, wait_clock):
        nc = self.nc
        drain_inst = nc.sync.drain()
        wait_clock.add_sem_waits(
            drain_inst.ins, tile.ScopedClock({None: tick_clock.global_clock})
        )
        nc.all_engine_barrier()
        popped = nc._tile_sem_poison_stack.pop()
        assert popped is self._sem_poison
        nc.clear_and_free_semaphores(list(self.sems.allocated().values()))

    tile.TileContext._drain_and_barrier = _drain_and_barrier


# ----------------------------------------------------------------------------
B, S, H = 2, 256, 768
F = 4 * H            # 3072
FQ = F // 4          # 768 FFN dims per core
KH = H // 128        # 6 H-chunks
NF = FQ // 128       # 6 FFN tiles per core
LR = 0.01
EPS = 1e-5
NSTEP = 5            # inverse product-formula steps (validated: rel err 4e-6)

F32 = mybir.dt.float32
F32R = mybir.dt.float32r
BF16 = mybir.dt.bfloat16
AF = mybir.ActivationFunctionType


def _f(ap):
    """View a float32r AP as plain fp32 for DVE/ACT consumers (same bits)."""
    return ap.bitcast(F32)

TRACE = False
TRACE_KW = {}


def build_bass():
    _slim_tile_exit()
    nc = bass.Bass()
    dt = F32
    xt = nc.dram_tensor("xt", [128, KH, S], F32R, kind="ExternalInput")        # x[b].T
    xtok = nc.dram_tensor("xtok", [S, H], dt, kind="ExternalInput")    # x[b]
    w0t = nc.dram_tensor("w0t", [128, KH, H], F32R, kind="ExternalInput")      # W_init.T
    gwt = nc.dram_tensor("gwt", [128, KH, FQ], BF16, kind="ExternalInput")     # gate_w[fsl].T
    uwt = nc.dram_tensor("uwt", [128, KH, FQ], BF16, kind="ExternalInput")     # up_w[fsl].T
    dwt = nc.dram_tensor("dwt", [128, NF, H], BF16, kind="ExternalInput")     # down_w[:,fsl].T
    gb = nc.dram_tensor("gb", [128, NF], dt, kind="ExternalInput")
    ub = nc.dram_tensor("ub", [128, NF], dt, kind="ExternalInput")
    b0 = nc.dram_tensor("b0", [H], dt, kind="ExternalInput")
    db = nc.dram_tensor("db", [H], dt, kind="ExternalInput")
    nw = nc.dram_tensor("nw", [H], dt, kind="ExternalInput")
    nbt = nc.dram_tensor("nbt", [H], dt, kind="ExternalInput")
    lowm = nc.dram_tensor("lowm", [128, 128], dt, kind="ExternalInput")
    upm = nc.dram_tensor("upm", [128, 128], dt, kind="ExternalInput")
    eye = nc.dram_tensor("eye", [128, 128], dt, kind="ExternalInput")
    z0 = nc.dram_tensor("z0", [128, 1], dt, kind="ExternalInput")
    z1 = nc.dram_tensor("z1", [128, 1], dt, kind="ExternalInput")
    z0q = nc.dram_tensor("z0q", [128, 1], dt, kind="ExternalInput")
    z1q = nc.dram_tensor("z1q", [128, 1], dt, kind="ExternalInput")
    out = nc.dram_tensor("out", [64, H], dt, kind="ExternalOutput")

    with tile.TileContext(nc) as tc:
        with tc.tile_pool(name="singles", bufs=1) as singles, \
             tc.tile_pool(name="work", bufs=2) as work, \
             tc.tile_pool(name="dram", bufs=1, space="DRAM") as dram:

            # --------------------------------------------------------------
            # Loads: ordered by when compute needs them (critical first).
            xt_sb = singles.tile([128, KH, S], F32R)
            nc.sync.dma_start(xt_sb[:], xt[:, :, :])
            lowm_sb = singles.tile([128, 128], F32)
            nc.sync.dma_start(lowm_sb[:], lowm[:, :])
            upm_sb = singles.tile([128, 128], F32)
            nc.sync.dma_start(upm_sb[:], upm[:, :])
            eye_sb = singles.tile([128, 128], F32)
            nc.sync.dma_start(eye_sb[:], eye[:, :])
            w0t_sb = singles.tile([128, KH, H], F32R)
            nc.sync.dma_start(w0t_sb[:], w0t[:, :, :])
            xn_sb = singles.tile([128, 2, H], F32)
            nc.vector.memset(xn_sb[:, 1, :], 0.0)
            nc.sync.dma_start(xn_sb[:, 0, :], xtok[1:129, :])
            nc.sync.dma_start(xn_sb[:127, 1, :], xtok[129:256, :])
            b0_bc = singles.tile([128, H], F32)
            nc.sync.dma_start(b0_bc[:], b0[:].partition_broadcast(128))
            db_bc = singles.tile([128, H], F32)
            nc.sync.dma_start(db_bc[:], db[:].partition_broadcast(128))
            z0_sb = singles.tile([128, 1], F32)
            nc.sync.dma_start(z0_sb[:], z0[:, :])
            z1_sb = singles.tile([128, 1], F32)
            nc.sync.dma_start(z1_sb[:], z1[:, :])
            z0q_sb = singles.tile([128, 1], F32)
            nc.sync.dma_start(z0q_sb[:], z0q[:, :])
            z1q_sb = singles.tile([128, 1], F32)
            nc.sync.dma_start(z1q_sb[:], z1q[:, :])
            gb_sb = singles.tile([128, NF], F32)
            nc.sync.dma_start(gb_sb[:], gb[:, :])
            ub_sb = singles.tile([128, NF], F32)
            nc.sync.dma_start(ub_sb[:], ub[:, :])
            nw_bc = singles.tile([64, H], F32)
            nc.sync.dma_start(nw_bc[:], nw[:].partition_broadcast(64))
            nb_bc = singles.tile([64, H], F32)
            nc.sync.dma_start(nb_bc[:], nbt[:].partition_broadcast(64))
            gwt_sb = singles.tile([128, KH, FQ], BF16)
            nc.sync.dma_start(gwt_sb[:], gwt[:, :, :])
            uwt_sb = singles.tile([128, KH, FQ], BF16)
            nc.sync.dma_start(uwt_sb[:], uwt[:, :, :])
            dwt_sb = singles.tile([128, NF, H], BF16)
            nc.sync.dma_start(dwt_sb[:], dwt[:, :, :])

            # warm up the collectives engine early with a tiny RS so the real
            # one at the end does not pay ncfw first-call startup
            warm_in = dram.tile([8, 16], F32)
            warm_out = dram.tile([1, 16], F32)
            warm_sb = work.tile([8, 16], F32, tag="warm", bufs=1)
            nc.vector.memset(warm_sb[:], 0.0)
            nc.sync.dma_start(warm_in[:, :], warm_sb[:])
            nc.gpsimd.collective_compute(
                "ReduceScatter",
                mybir.AluOpType.add,
                replica_groups=[[0, 1, 2, 3, 4, 5, 6, 7]],
                ins=[warm_in[:, :]],
                outs=[warm_out[:, :]],
            )

            lr_bias = singles.tile([128, 1], F32)
            nc.vector.memset(lr_bias[:], LR)
            lrG_A = singles.tile([128, 256], F32R)
            lrG_B = singles.tile([128, 128], F32R)
            Qlo = singles.tile([128, 2, 128], F32R)
            Qup = singles.tile([128, 2, 128], F32R)
            Tt = singles.tile([128, 2, 128], F32R)
            R = singles.tile([128, 2, H], F32R)
            d_tok = singles.tile([128, 2, H], F32R)
            predT = singles.tile([128, KH, S], BF16)
            pred_q = singles.tile([128, 2, H], F32)
            pred_q0 = singles.tile([128, 2, H], F32)
            pred_q1 = singles.tile([128, 2, H], F32)

            with tc.tile_pool(name="psum_p1", bufs=3, space="PSUM") as psum_p1:
                # ----------------------------------------------------------
                # Phase 1a: Gram blocks -> lr*(G+1)
                ps_ga = psum_p1.tile([128, 256], F32, tag="p1")
                for k in range(KH):
                    nc.tensor.matmul(ps_ga[:], (xt_sb[:, k, 0:128]),
                                     (xt_sb[:, k, :]),
                                     start=(k == 0), stop=(k == KH - 1))
                nc.scalar.activation(lrG_A[:], ps_ga[:], AF.Identity,
                                     bias=lr_bias[:], scale=LR)
                ps_gb = psum_p1.tile([128, 128], F32, tag="p1")
                for k in range(KH):
                    nc.tensor.matmul(ps_gb[:], xt_sb[:, k, 128:256],
                                     xt_sb[:, k, 128:256],
                                     start=(k == 0), stop=(k == KH - 1))
                nc.scalar.activation(lrG_B[:], ps_gb[:], AF.Identity,
                                     bias=lr_bias[:], scale=LR)

                nc.vector.tensor_mul(Qlo[:, 0, :], _f(lrG_A[:, 0:128]), lowm_sb[:])
                nc.vector.tensor_mul(Qup[:, 0, :], _f(lrG_A[:, 0:128]), upm_sb[:])
                nc.vector.tensor_mul(Qlo[:, 1, :], _f(lrG_B[:]), lowm_sb[:])
                nc.vector.tensor_mul(Qup[:, 1, :], _f(lrG_B[:]), upm_sb[:])
                nc.vector.tensor_sub(Tt[:, 0, :], eye_sb[:], _f(Qup[:, 0, :]))
                nc.vector.tensor_sub(Tt[:, 1, :], eye_sb[:], _f(Qup[:, 1, :]))

                # Phase 1b: Tt_c = inv(I + U_c), product formula, both chunks
                for s in range(1, NSTEP + 1):
                    ps_l = psum_p1.tile([128, 2, 128], F32, tag="p1")
                    ps_u = psum_p1.tile([128, 2, 128], F32, tag="p1")
                    for c in range(2):
                        nc.tensor.matmul(ps_l[:, c, :], Qup[:, c, :], Qlo[:, c, :],
                                         start=True, stop=True)
                        if s < NSTEP:
                            nc.tensor.matmul(ps_u[:, c, :], Qlo[:, c, :],
                                             Qup[:, c, :], start=True, stop=True)
                    nc.vector.tensor_copy(Qlo[:], ps_l[:])
                    if s < NSTEP:
                        nc.vector.tensor_copy(Qup[:], ps_u[:])
                    ps_t = psum_p1.tile([128, 2, 128], F32, tag="p1")
                    for c in range(2):
                        nc.tensor.matmul(ps_t[:, c, :], Qlo[:, c, :], Tt[:, c, :],
                                         start=True, stop=True)
                    nc.vector.tensor_add(Tt[:], _f(Tt[:]), ps_t[:])

                # ----------------------------------------------------------
                # Phase 1c: P0 (token-major), R = P0 + b0 - xnext
                for c in range(2):
                    ps_p0 = psum_p1.tile([128, H], F32, tag="p1")
                    for k in range(KH):
                        nc.tensor.matmul(ps_p0[:, 0:512],
                                         (xt_sb[:, k, bass.ts(c, 128)]),
                                         (w0t_sb[:, k, 0:512]),
                                         start=(k == 0), stop=(k == KH - 1))
                        nc.tensor.matmul(ps_p0[:, 512:768],
                                         (xt_sb[:, k, bass.ts(c, 128)]),
                                         (w0t_sb[:, k, 512:768]),
                                         start=(k == 0), stop=(k == KH - 1))
                    nc.vector.tensor_sub(R[:, c, :], ps_p0[:], xn_sb[:, c, :])
                    nc.vector.tensor_add(R[:, c, :], R[:, c, :], b0_bc[:])

                # Phase 1d: solve
                ps_d1 = psum_p1.tile([128, H], F32, tag="p1")
                nc.tensor.matmul(ps_d1[:, 0:512], (Tt[:, 0, :]),
                                 (R[:, 0, 0:512]), start=True, stop=True)
                nc.tensor.matmul(ps_d1[:, 512:768], (Tt[:, 0, :]),
                                 (R[:, 0, 512:768]), start=True, stop=True)
                nc.vector.tensor_copy(d_tok[:, 0, :], ps_d1[:])
                ps_v = psum_p1.tile([128, H], F32, tag="p1")
                nc.tensor.matmul(ps_v[:, 0:512], (lrG_A[:, 128:256]),
                                 (d_tok[:, 0, 0:512]), start=True, stop=True)
                nc.tensor.matmul(ps_v[:, 512:768], (lrG_A[:, 128:256]),
                                 (d_tok[:, 0, 512:768]), start=True, stop=True)
                nc.vector.tensor_sub(R[:, 1, :], _f(R[:, 1, :]), ps_v[:])
                ps_d2 = psum_p1.tile([128, H], F32, tag="p1")
                nc.tensor.matmul(ps_d2[:, 0:512], (Tt[:, 1, :]),
                                 (R[:, 1, 0:512]), start=True, stop=True)
                nc.tensor.matmul(ps_d2[:, 512:768], (Tt[:, 1, :]),
                                 (R[:, 1, 512:768]), start=True, stop=True)
                nc.vector.tensor_copy(d_tok[:, 1, :], ps_d2[:])

                # pred_q{0,1} = (pred + db)*0.25*z{0,1} (token-major)
                for c in range(2):
                    nc.vector.tensor_add(pred_q[:, c, :], _f(d_tok[:, c, :]),
                                         xn_sb[:, c, :])
                    nc.vector.tensor_add(pred_q[:, c, :], pred_q[:, c, :], db_bc[:])
                    nc.scalar.activation(pred_q0[:, c, :], pred_q[:, c, :],
                                         AF.Identity, scale=z0q_sb[:])
                    nc.scalar.activation(pred_q1[:, c, :], pred_q[:, c, :],
                                         AF.Identity, scale=z1q_sb[:])

                # feat-major predT = (R^T Tt) + shifted x^T
                for c in range(2):
                    for k in range(KH):
                        ps_dt = psum_p1.tile([128, 128], F32, tag="p1")
                        nc.tensor.matmul(ps_dt[:], R[:, c, bass.ts(k, 128)],
                                         Tt[:, c, :], start=True, stop=True)
                        if c == 0:
                            nc.vector.tensor_add(predT[:, k, 0:128], ps_dt[:],
                                                 _f(xt_sb[:, k, 1:129]))
                        else:
                            nc.vector.tensor_add(predT[:, k, 128:255],
                                                 ps_dt[:, 0:127],
                                                 _f(xt_sb[:, k, 129:256]))
                            nc.vector.tensor_copy(predT[:, k, 255:256],
                                                  ps_dt[:, 127:128])

            # --------------------------------------------------------------
            # Phase 2: MLP (feat-major gate/up, token-major down partials)
            with tc.tile_pool(name="psum_dn", bufs=1, space="PSUM") as psum_dn, \
                 tc.tile_pool(name="psum_mlp", bufs=4, space="PSUM") as psum_mlp:
                ps_dn0 = psum_dn.tile([128, H], F32, tag="dn0")
                ps_dn1 = psum_dn.tile([128, H], F32, tag="dn1")
                ps_dns = (ps_dn0, ps_dn1)
                # keep the PE array busy through the serial solve tail so the
                # HAM clock gate stays at full rate when the MLP burst starts
                ps_w = psum_mlp.tile([128, S], F32, tag="mlp")
                for w in range(12):
                    nc.tensor.matmul(ps_w[:], gwt_sb[:, w % KH, 0:128],
                                     gwt_sb[:, w % KH, 0:256].bitcast(BF16),
                                     start=(w == 0), stop=(w == 11),
                                     skip_group_check=True)
                for j in range(NF):
                    ps_g = psum_mlp.tile([128, S], F32, tag="mlp")
                    ps_u = psum_mlp.tile([128, S], F32, tag="mlp")
                    for k in range(KH):
                        nc.tensor.matmul(ps_g[:], (gwt_sb[:, k, bass.ts(j, 128)]),
                                         (predT[:, k, :]),
                                         start=(k == 0), stop=(k == KH - 1))
                    for k in range(KH):
                        nc.tensor.matmul(ps_u[:], (uwt_sb[:, k, bass.ts(j, 128)]),
                                         (predT[:, k, :]),
                                         start=(k == 0), stop=(k == KH - 1))
                    sig = work.tile([128, S], F32, tag="sig")
                    nc.scalar.activation(sig[:], ps_g[:], AF.Sigmoid,
                                         bias=gb_sb[:, j : j + 1], scale=1.0)
                    gtd = work.tile([128, S], BF16, tag="gtd")
                    nc.vector.tensor_scalar(gtd[:], ps_u[:], ub_sb[:, j : j + 1],
                                            None, mybir.AluOpType.add)
                    nc.vector.tensor_mul(gtd[:], gtd[:], sig[:])
                    for m in range(2):
                        nc.tensor.matmul(ps_dns[m][:, 0:512],
                                         (gtd[:, bass.ts(m, 128)]),
                                         (dwt_sb[:, j, 0:512]),
                                         start=(j == 0), stop=(j == NF - 1))
                        nc.tensor.matmul(ps_dns[m][:, 512:768],
                                         (gtd[:, bass.ts(m, 128)]),
                                         (dwt_sb[:, j, 512:768]),
                                         start=(j == 0), stop=(j == NF - 1))

                # partial + (pred + db)/4  -> DRAM for the exchange.
                # 8-rank ReduceScatter over the full flattened token space
                # [B*S, H]: core c's tokens are exactly shard c (64c..64c+64).
                # Each core contributes its batch's partials in that batch's
                # row-half and zeros in the other; which half is selected by
                # the 0/1 input scalars z0/z1 (program stays SPMD-uniform).
                HP = 1024  # pad H so the 2S x HP bf16 buffer exceeds the
                # ~1MB algorithm crossover (mesh below is much slower here)
                rs_in = dram.tile([2 * S, HP], BF16)
                rs_out = dram.tile([64, HP], BF16)
                dp0 = work.tile([128, 2, HP], BF16, tag="dp0", bufs=1)
                dp1 = work.tile([128, 2, HP], BF16, tag="dp1", bufs=1)
                nc.vector.memset(dp0[:, :, H:], 0.0)
                nc.vector.memset(dp1[:, :, H:], 0.0)
                for m in range(2):
                    nc.scalar.activation(dp0[:, m, :H], ps_dns[m][:], AF.Identity,
                                         scale=z0_sb[:])
                    nc.vector.tensor_add(dp0[:, m, :H], dp0[:, m, :H],
                                         pred_q0[:, m, :])
                    nc.scalar.activation(dp1[:, m, :H], ps_dns[m][:], AF.Identity,
                                         scale=z1_sb[:])
                    nc.vector.tensor_add(dp1[:, m, :H], dp1[:, m, :H],
                                         pred_q1[:, m, :])
                    nc.sync.dma_start(rs_in[bass.ts(m, 128), :], dp0[:, m, :])
                    nc.sync.dma_start(rs_in[bass.ds(S + m * 128, 128), :],
                                      dp1[:, m, :])

            nc.gpsimd.collective_compute(
                "ReduceScatter",
                mybir.AluOpType.add,
                replica_groups=[[0, 1, 2, 3, 4, 5, 6, 7]],
                ins=[rs_in[:, :]],
                outs=[rs_out[:, :]],
            )

            # --------------------------------------------------------------
            # Phase 3: LayerNorm on the summed 64-token shard
            acc_h = work.tile([64, HP], BF16, tag="acc_h", bufs=1)
            nc.sync.dma_start(acc_h[:], rs_out[:, :])
            acc = work.tile([64, H], F32, tag="acc", bufs=1)
            nc.gpsimd.tensor_copy(out=acc[:], in_=acc_h[:, :H])

            nstats = nc.vector.BN_STATS_DIM
            stats = work.tile([64, 3, nstats], F32, tag="stats", bufs=1)
            accr = acc[:].rearrange("t (n d) -> t n d", d=256)
            for g in range(3):
                nc.vector.bn_stats(out=stats[:, g, :], in_=accr[:, g, :])
            mv = work.tile([64, nc.vector.BN_AGGR_DIM], F32, tag="mv", bufs=1)
            nc.vector.bn_aggr(out=mv[:], in_=stats[:])
            eps_t = work.tile([64, 1], F32, tag="eps", bufs=1)
            nc.vector.memset(eps_t[:], EPS)
            rstd = work.tile([64, 1], F32, tag="rstd", bufs=1)
            nc.scalar.activation(out=rstd[:], in_=mv[:, 1:2], func=AF.Sqrt,
                                 bias=eps_t[:], scale=1.0)
            nc.vector.reciprocal(out=rstd[:], in_=rstd[:])
            nc.vector.tensor_scalar(acc[:], acc[:], mv[:, 0:1], rstd[:],
                                    mybir.AluOpType.subtract,
                                    mybir.AluOpType.mult)
            nc.vector.tensor_mul(acc[:], acc[:], nw_bc[:])
            nc.vector.tensor_add(acc[:], acc[:], nb_bc[:])
            nc.sync.dma_start(out[:, :], acc[:])
    _split_multi_waits(nc)
    return nc


# ----------------------------------------------------------------------------
_NC_CACHE = None


def _get_nc():
    global _NC_CACHE
    if _NC_CACHE is None:
        _NC_CACHE = build_bass()
    return _NC_CACHE


def kernel(x, W_init, b_init, gate_w, gate_b, up_w, up_b, down_w, down_b,
           norm_w, norm_b):
    x = np.asarray(x, np.float32)
    W_init = np.asarray(W_init, np.float32)
    b_init = np.asarray(b_init, np.float32)
    gate_w = np.asarray(gate_w, np.float32)
    gate_b = np.asarray(gate_b, np.float32)
    up_w = np.asarray(up_w, np.float32)
    up_b = np.asarray(up_b, np.float32)
    down_w = np.asarray(down_w, np.float32)
    down_b = np.asarray(down_b, np.float32)
    norm_w = np.asarray(norm_w, np.float32)
    norm_b = np.asarray(norm_b, np.float32)

    nc = _get_nc()

    def shuf(a):
        # [O*128, N] -> [128, O, N] with partition p holding rows {o*128+p}
        o = a.shape[0] // 128
        return np.ascontiguousarray(a.reshape(o, 128, -1).transpose(1, 0, 2))

    lowmask = np.tril(np.ones((128, 128), np.float32), -1)
    upmask = np.triu(np.ones((128, 128), np.float32), 1)
    eye = np.eye(128, dtype=np.float32)
    w0t = shuf(W_init.T)

    in_maps = []
    for c in range(8):
        b, q = c // 4, c % 4
        fsl = slice(q * FQ, (q + 1) * FQ)
        in_maps.append({
            "xt": shuf(x[b].T),
            "xtok": np.ascontiguousarray(x[b]),
            "w0t": w0t,
            "gwt": shuf(gate_w[fsl].T.astype(ml_dtypes.bfloat16)),
            "uwt": shuf(up_w[fsl].T.astype(ml_dtypes.bfloat16)),
            "dwt": shuf(down_w[:, fsl].T.astype(ml_dtypes.bfloat16)),
            "gb": np.ascontiguousarray(gate_b[fsl].reshape(NF, 128).T),
            "ub": np.ascontiguousarray(up_b[fsl].reshape(NF, 128).T),
            "b0": b_init,
            "db": down_b,
            "nw": norm_w,
            "nbt": norm_b,
            "z0": np.full((128, 1), 1.0 if b == 0 else 0.0, np.float32),
            "z1": np.full((128, 1), 1.0 if b == 1 else 0.0, np.float32),
            "z0q": np.full((128, 1), 0.25 if b == 0 else 0.0, np.float32),
            "z1q": np.full((128, 1), 0.25 if b == 1 else 0.0, np.float32),
            "lowm": lowmask,
            "upm": upmask,
            "eye": eye,
        })

    res = bass_utils.run_bass_kernel_spmd(
        nc, in_maps, core_ids=list(range(8)), trace=TRACE, **TRACE_KW
    )
    if TRACE:
        kernel.last_exec_time_ns = res.exec_time_ns

    out = np.empty((B, S, H), np.float32)
    for c in range(8):
        b, q = c // 4, c % 4
        out[b, q * 64 : (q + 1) * 64, :] = res.results[c]["out"]
    return out


kernel.last_exec_time_ns = None
